# revision 22
# baseline (speedup 1.0000x reference)
"""Trainium2 8-core Bass kernel for the AntiviralGNN problem.

3-layer GATv2 (edge features, self-loops with mean fill) + mean/max graph
pooling + MLP classifier.  Nodes/edges are partitioned across 8 NeuronCores
by graph blocks (32 graphs per core); per layer each core computes its
xl = h@Wl+bl slice, all-gathers xl (bf16), then processes its incoming
edges: gathers xl[src] rows with dma_gather, reconstructs
z = leaky_relu(xl[src]+xr[dst]+ea@We) via TensorE matmuls (one-hot dst masks),
does the edge softmax without max-subtraction (alpha is provably in [-1,1])
and scatter-adds messages back to nodes with mask matmuls into PSUM.
"""

import os
import sys
import time

sys.path.insert(0, '/opt/trn_rl_repo')

import numpy as np

import concourse.bass as bass
import concourse.bacc as bacc
import concourse.mybir as mybir
import concourse.tile as tile
from concourse import bass_utils

BF16 = mybir.dt.np(mybir.dt.bfloat16)
FP8 = mybir.dt.np(mybir.dt.float8e4)
F8 = mybir.dt.float8e4
F32 = mybir.dt.float32
BF = mybir.dt.bfloat16
I16 = mybir.dt.int16
AF = mybir.ActivationFunctionType
OP = mybir.AluOpType

NCORES = 8
N, E, G = 20000, 320000, 256
FN, FE, HID, H, C = 64, 16, 256, 4, 64
GPC = G // NCORES          # graphs per core
EPS = 1e-5
P = 128


def _log(msg):
    print(f"[kernel +{time.time() - _T0:8.1f}s] {msg}", file=sys.stderr, flush=True)


_T0 = time.time()


def _chunk_bounds(nt_cnt):
    """Node-tile boundaries of the all-gather chunks (cumulative), unbalanced
    ~[40%, 30%, 20%, 10%] so the final (critical-path) chunk is small."""
    if nt_cnt < 6:
        return list(range(nt_cnt + 1))
    r5 = 1
    r4 = max(1, round(nt_cnt * 0.15))
    r3 = max(1, round(nt_cnt * 0.20))
    r2 = max(1, round(nt_cnt * 0.25))
    r1 = nt_cnt - r2 - r3 - r4 - r5
    return [0, r1, r1 + r2, r1 + r2 + r3, r1 + r2 + r3 + r4, nt_cnt]


# ---------------------------------------------------------------- host prep
def _preprocess(inputs):
    x = inputs['x'].astype(np.float32)
    src, dst = inputs['edge_index'][0].astype(np.int64), inputs['edge_index'][1].astype(np.int64)
    batch = inputs['batch'].astype(np.int64)
    ea = inputs['edge_attr'].astype(np.float32)

    gstart = np.searchsorted(batch, np.arange(G + 1))
    gsize = np.diff(gstart)
    maxg = int(gsize.max())
    gslot = max(80, -(-maxg // 4) * 4)          # slots per graph, %4==0
    npad = GPC * gslot                           # padded nodes per core
    assert npad % P == 0
    nt_cnt = npad // P

    # global padded slot id per node.  All-gather chunks are UNBALANCED
    # (large first, small last) so the last chunk's collective - the only one
    # exposed on the critical path - is short.
    owner = batch // GPC
    slot_local = (batch % GPC) * gslot + (np.arange(N) - gstart[batch])
    cb = _chunk_bounds(nt_cnt)
    bs = np.array(cb) * P                        # chunk slot boundaries
    _k = np.searchsorted(bs, slot_local, 'right') - 1
    csz = bs[_k + 1] - bs[_k]
    gid = NCORES * bs[_k] + owner * csz + (slot_local - bs[_k])

    # self-loop attrs (mean of incoming)
    deg = np.zeros(N)
    np.add.at(deg, dst, 1.0)
    loop_attr = np.zeros((N, FE), np.float32)
    np.add.at(loop_attr, dst, ea)
    loop_attr /= np.maximum(deg, 1.0)[:, None].astype(np.float32)

    src2 = np.concatenate([src, np.arange(N)])
    dst2 = np.concatenate([dst, np.arange(N)])
    ea2 = np.concatenate([ea, loop_attr], axis=0)

    e_owner = owner[dst2]
    e_slot = slot_local[dst2]
    e_nt = e_slot // P
    e_rel = e_slot % P
    e_bin = e_owner * nt_cnt + e_nt

    order = np.argsort(e_bin, kind='stable')
    bin_cnt = np.bincount(e_bin, minlength=NCORES * nt_cnt)
    te = int(-(-bin_cnt.max() // P))             # edge tiles per node tile
    te += te % 2                                 # pair-batched loops need even te
    epp = te * P                                 # padded edges per node tile

    # position of each edge inside its padded bin
    bin_of = e_bin[order]
    start = np.zeros(NCORES * nt_cnt, np.int64)
    start[1:] = np.cumsum(bin_cnt)[:-1]
    pos_in_bin = np.arange(len(order)) - start[bin_of]
    flat = bin_of * epp + pos_in_bin             # position in padded global layout

    tot = NCORES * nt_cnt * epp
    src_g = np.zeros(tot, np.int64)              # gather ids (pad -> 0)
    rel_g = np.full(tot, 200.0, np.float32)      # dst_rel (pad -> 200)
    ea_g = np.zeros((tot, FE), np.float32)
    src_g[flat] = gid[src2[order]]
    rel_g[flat] = e_rel[order]
    ea_g[flat] = ea2[order]

    src_g = src_g.reshape(NCORES, nt_cnt, epp)
    rel_g = rel_g.reshape(NCORES, nt_cnt, epp)
    ea_g = ea_g.reshape(NCORES, nt_cnt, epp, FE)

    per_core = []
    for c in range(NCORES):
        # gather indices: per node tile, idx j -> [j%16, j//16]; replicated to
        # 128 partitions (one copy per Q7 core)
        for_nt = src_g[c].reshape(nt_cnt, epp)
        iw = for_nt.reshape(nt_cnt, epp // 16, 16).transpose(2, 0, 1).reshape(16, -1).astype(np.int16)
        idx_w = np.tile(iw, (8, 1))

        # ef = ea @ We per layer, packed for per-pair DMA [3, nt, te//2, 128, 512]
        WeT = inputs['conv_We'].astype(np.float32)          # [3, 16, 256]
        ef = np.einsum('nef,lfc->lnec', ea_g[c].reshape(nt_cnt, epp, FE), WeT)
        ef = ef.reshape(3, nt_cnt, te // 2, 2, P, HID).transpose(0, 1, 2, 4, 3, 5)
        ef = ef.reshape(3, nt_cnt, te // 2, P, 2 * HID).astype(BF16)

        # m0T [nt, 128, te*128]: (n == rel)
        rel_tiles = rel_g[c].reshape(nt_cnt, epp)
        m0T = (np.arange(P)[None, :, None] == rel_tiles[:, None, :]).astype(np.float32)
        # m0e [nt, 128e, te*128]: row e, col et*128+n = (rel[et*128+e] == n)
        rel_etp = rel_g[c].reshape(nt_cnt, te, P)
        m0e = (rel_etp[:, :, :, None] == np.arange(P)[None, None, None, :])
        m0e = m0e.transpose(0, 2, 1, 3).reshape(nt_cnt, P, epp).astype(np.float32)

        # node mask [128, nt]
        msk = np.zeros(npad, np.float32)
        msk[slot_local[owner == c]] = 1.0
        msk = msk.reshape(nt_cnt, P).T.copy()

        # x transposed with ones row [65, npad]
        xT = np.zeros((FN + 1, npad), np.float32)
        xT[:FN, slot_local[owner == c]] = x[owner == c].T
        xT[FN, :] = 1.0
        xT = xT.astype(BF16)

        cnt = gsize[c * GPC:(c + 1) * GPC]
        rc = np.zeros((P, 1), np.float32)
        rc[:GPC, 0] = 1.0 / cnt
        per_core.append(dict(idx=idx_w, ef=ef,
                             m0T=m0T.astype(FP8), m0e=m0e.astype(FP8),
                             mask=msk, xT=xT, recip_cnt=rc))

    meta = dict(gslot=gslot, npad=npad, nt=nt_cnt, te=te)
    return per_core, meta, gid


def _fold_params(inputs):
    """Host-side parameter folding -> device tensors (shared across cores)."""
    p = {}
    encW = np.concatenate([inputs['enc_W'], inputs['enc_b'][None, :]], axis=0)
    p['encW'] = encW.astype(BF16)                                   # [65, 256]

    Wl = inputs['conv_Wl'].reshape(3, 2, P, HID)                    # [3,2,128,256]
    Wr = inputs['conv_Wr'].reshape(3, 2, P, HID)
    Wlr = np.concatenate([Wl, Wr], axis=3)                          # [3,2,128,512]
    p['Wlr'] = Wlr.transpose(2, 0, 1, 3).astype(np.float16)         # [128,3,2,512]
    p['We'] = inputs['conv_We'].transpose(1, 0, 2).astype(BF16)     # [16,3,256]

    att = inputs['conv_att'].reshape(3, 1, HID)
    att2 = np.concatenate([att, att], axis=2)
    p['attb2'] = np.tile(att2, (1, P, 1)).transpose(1, 0, 2).astype(BF16)  # [128,3,512]

    def bc(v):   # [3,256] -> [128,3,256]
        return np.tile(v[:, None, :], (1, P, 1)).transpose(1, 0, 2).astype(np.float32)

    p['blbc'] = bc(inputs['conv_bl'])
    p['brbc'] = bc(inputs['conv_br'])
    rs = 1.0 / np.sqrt(inputs['bn_var'] + EPS)
    sc = rs * inputs['bn_gamma']
    sh = (inputs['conv_bias'] - inputs['bn_mean']) * sc + inputs['bn_beta']
    p['bnsc'] = bc(sc)
    p['bnsh'] = bc(sh)

    p['identbf'] = np.eye(P, dtype=np.float32).astype(BF16)
    p['identf'] = np.eye(P, dtype=np.float32)

    p['W1'] = inputs['cls_W1'].reshape(4, P, HID).transpose(1, 0, 2).astype(np.float32)  # [128,4,256]
    p['W2'] = inputs['cls_W2'].reshape(2, P, HID // 2).transpose(1, 0, 2).astype(np.float32)  # [128,2,128]
    p['W3'] = inputs['cls_W3'].astype(np.float32)                   # [128,1]

    s1 = inputs['cls_g1'] / np.sqrt(inputs['cls_v1'] + EPS)
    t1 = (inputs['cls_b1'] - inputs['cls_m1']) * s1 + inputs['cls_bt1']
    s2 = inputs['cls_g2'] / np.sqrt(inputs['cls_v2'] + EPS)
    t2 = (inputs['cls_b2'] - inputs['cls_m2']) * s2 + inputs['cls_bt2']
    p['s1bc'] = np.tile(s1, (P, 1)).astype(np.float32)              # [128,256]
    p['t1bc'] = np.tile(t1, (P, 1)).astype(np.float32)
    p['s2bc'] = np.tile(s2, (P, 1)).astype(np.float32)              # [128,128]
    p['t2bc'] = np.tile(t2, (P, 1)).astype(np.float32)
    p['b3'] = np.tile(inputs['cls_b3'].reshape(1, 1), (P, 1)).astype(np.float32)
    return p


# ---------------------------------------------------------------- device graph
def _build(meta, stage='full'):
    npad, NT, TE = meta['npad'], meta['nt'], meta['te']
    EPP = TE * P
    nc = bacc.Bacc("TRN2", target_bir_lowering=False, debug=False, num_devices=NCORES,
                   num_swdge_queues=2)

    dram = {}

    def din(name, shape, dt):
        dram[name] = nc.dram_tensor(name, shape, dt, kind="ExternalInput")
        return dram[name]

    din('xT', [FN + 1, npad], BF)
    din('idx', [P, NT * TE * 8], I16)
    din('ef', [3, NT, TE // 2, P, 2 * HID], BF)
    din('m0T', [NT, P, EPP], F8)
    din('m0e', [NT, P, EPP], F8)
    din('mask', [P, NT], F32)
    din('recip_cnt', [P, 1], F32)
    din('encW', [FN + 1, HID], BF)
    din('Wlr', [P, 3, 2, 2 * HID], mybir.dt.float16)
    din('We', [FE, 3, HID], BF)
    din('attb2', [P, 3, 2 * HID], BF)
    din('blbc', [P, 3, HID], F32)
    din('brbc', [P, 3, HID], F32)
    din('bnsc', [P, 3, HID], F32)
    din('bnsh', [P, 3, HID], F32)
    din('identbf', [P, P], BF)
    din('identf', [P, P], F32)
    din('W1', [P, 4, HID], F32)
    din('W2', [P, 2, HID // 2], F32)
    din('W3', [P, 1], F32)
    din('s1bc', [P, HID], F32)
    din('t1bc', [P, HID], F32)
    din('s2bc', [P, HID // 2], F32)
    din('t2bc', [P, HID // 2], F32)
    din('b3', [P, 1], F32)
    out_d = nc.dram_tensor("out", [GPC, 1], F32, kind="ExternalOutput")


    with tile.TileContext(nc) as tc:
        with (
            tc.tile_pool(name="const", bufs=1) as cp,
            tc.tile_pool(name="state", bufs=1) as st,
            tc.tile_pool(name="work", bufs=2) as wk,
            tc.tile_pool(name="pv", bufs=2, space="PSUM") as pv,
            tc.tile_pool(name="pacc", bufs=2, space="PSUM") as pacc,
            tc.tile_pool(name="pprod", bufs=2, space="PSUM") as pprod,
            tc.tile_pool(name="dramp", bufs=2, space="DRAM") as dp,
        ):
            # ---- load constants
            cs = {}
            for name in ['xT', 'idx', 'mask', 'recip_cnt', 'encW', 'Wlr', 'We', 'attb2', 'blbc', 'brbc', 'bnsc', 'bnsh',
                         'identbf', 'identf', 'W1', 'W2', 'W3', 's1bc',
                         't1bc', 's2bc', 't2bc', 'b3']:
                d = dram[name]
                t = cp.tile(list(d.shape), d.dtype, name=f"c_{name}")
                nc.sync.dma_start(out=t[:], in_=d.ap())
                cs[name] = t

            # ---- state
            h_a = st.tile([P, NT, HID], F32, name="h_a")
            h_b = st.tile([P, NT, HID], F32, name="h_b")
            hT_a = st.tile([P, 2, npad], F32, name="hT_a")
            hTb = st.tile([P, 2, npad], mybir.dt.float16, name="hTb")
            xr_all = st.tile([P, NT, HID], BF, name="xr_all")

            def make_hT(hT, h_cur, nt):
                for cc in range(2):
                    tp = pprod.tile([P, P], F32, tag="tp", bufs=2)
                    nc.tensor.transpose(tp[:], h_cur[:, nt, cc * P:(cc + 1) * P], cs['identf'][:])
                    nc.scalar.copy(hT[:, cc, nt * P:(nt + 1) * P], tp[:])
                    nc.scalar.copy(hTb[:, cc, nt * P:(nt + 1) * P], tp[:])

            # chunked all-gather, unbalanced (large->small) chunks; tile bounds
            CB = _chunk_bounds(NT)
            NCH = len(CB) - 1
            CH_ENDS = {CB[k + 1]: k for k in range(NCH)}
            hT = hT_a
            xr_b = st.tile([P, NT, HID], BF, name="xr_b")
            xr_ab = [xr_all, xr_b]

            def produce(li, nt, xl_bounce):
                pp = pprod.tile([P, 2 * HID], F32, tag="prod")
                for cc in range(2):
                    nc.tensor.matmul(pp[:], hTb[:, cc, nt * P:(nt + 1) * P],
                                     cs['Wlr'][:, li, cc, :],
                                     start=(cc == 0), stop=(cc == 1))
                xl_sb = wk.tile([P, HID], BF, tag="xlsb")
                nc.vector.scalar_tensor_tensor(
                    xl_sb[:], pp[:, 0:HID], 1.0, cs['blbc'][:, li, :], OP.mult, OP.add)
                nc.sync.dma_start(out=xl_bounce[nt * P:(nt + 1) * P, :], in_=xl_sb[:])
                nc.vector.scalar_tensor_tensor(
                    xr_ab[li % 2][:, nt, :], pp[:, HID:2 * HID], 1.0,
                    cs['brbc'][:, li, :], OP.mult, OP.add)

            def chunk_collective(xl_bounce, xl_full, ck):
                rs, re = CB[ck] * P, CB[ck + 1] * P
                nc.gpsimd.collective_compute(
                    "AllGather", OP.bypass,
                    replica_groups=[list(range(NCORES))],
                    ins=[xl_bounce[rs:re, :].opt()],
                    outs=[xl_full[NCORES * rs:NCORES * re, :].opt()],
                )

            xlb = [None] * 3
            xlf = [None] * 3
            xlb[0] = dp.tile([npad, HID], BF, tag="xlb", name="xlb0")
            xlf[0] = dp.tile([NCORES * npad, HID], BF, tag="xlf", name="xlf0")

            # ---- encoder (+ layer-0 production, chunk-collectives)
            with nc.named_scope("encoder"):
                for nt in range(NT):
                    pp = pprod.tile([P, HID], F32, tag="prod")
                    nc.tensor.matmul(pp[:], cs['xT'][:, nt * P:(nt + 1) * P],
                                     cs['encW'][:], start=True, stop=True)
                    nc.scalar.activation(h_a[:, nt, :], pp[:], AF.Relu,
                                         scale=cs['mask'][:, nt:nt + 1])
                    make_hT(hT, h_a, nt)
                    produce(0, nt, xlb[0])
                    if (nt + 1) in CH_ENDS:
                        chunk_collective(xlb[0], xlf[0], CH_ENDS[nt + 1])

            h_cur, h_nxt = h_a, h_b

            # ---- layers
            for li in range(3):
                if li < 2:
                    xlb[li + 1] = dp.tile([npad, HID], BF, tag="xlb",
                                          name=f"xlb{li + 1}")
                    xlf[li + 1] = dp.tile([NCORES * npad, HID], BF, tag="xlf",
                                          name=f"xlf{li + 1}")
                xl_full = xlf[li]
                xr_rd = xr_ab[li % 2]
                pend_ep = []

                def epilogue(li, nt, acc):
                    den = wk.tile([P, H], F32, tag="den")
                    nc.vector.tensor_scalar(den[:], acc[:, HID:HID + 4], 1e-30,
                                            None, OP.max)
                    rec = wk.tile([P, H], F32, tag="rec")
                    nc.vector.reciprocal(rec[:], den[:])
                    hc = wk.tile([P, HID], F32, tag="ep", bufs=2)
                    for hh in range(H):
                        nc.scalar.activation(hc[:, hh * C:(hh + 1) * C],
                                             acc[:, hh * C:(hh + 1) * C],
                                             AF.Copy, scale=rec[:, hh:hh + 1])
                    t1 = wk.tile([P, HID], F32, tag="ep", bufs=2)
                    nc.vector.tensor_tensor(t1[:], hc[:], cs['bnsc'][:, li, :], OP.mult)
                    t2 = wk.tile([P, HID], F32, tag="ep", bufs=2)
                    nc.vector.scalar_tensor_tensor(
                        t2[:], t1[:], 1.0, cs['bnsh'][:, li, :], OP.mult, OP.add)
                    t3 = wk.tile([P, HID], F32, tag="ep", bufs=2)
                    nc.scalar.activation(t3[:], t2[:], AF.Relu)
                    nc.vector.scalar_tensor_tensor(
                        h_nxt[:, nt, :], t3[:], cs['mask'][:, nt:nt + 1],
                        h_cur[:, nt, :], OP.mult, OP.add)
                    make_hT(hT, h_nxt, nt)
                    if li < 2:
                        produce(li + 1, nt, xlb[li + 1])
                        if (nt + 1) in CH_ENDS:
                            chunk_collective(xlb[li + 1], xlf[li + 1],
                                             CH_ENDS[nt + 1])
                with nc.named_scope(f"layer{li}_edges"):
                    for nt in range(NT):
                        gbuf = wk.tile([P, TE, HID], BF, tag="gbuf", bufs=3)
                        th = TE // 2
                        nc.gpsimd.dma_gather(
                            gbuf[:, 0:th, :], xl_full[:],
                            cs['idx'][:, nt * TE * 8:nt * TE * 8 + th * 8],
                            th * P, th * P, HID, single_packet=False, queue_num=0)
                        nc.gpsimd.dma_gather(
                            gbuf[:, th:TE, :], xl_full[:],
                            cs['idx'][:, nt * TE * 8 + th * 8:(nt + 1) * TE * 8],
                            (TE - th) * P, (TE - th) * P, HID, single_packet=False,
                            queue_num=1)
                        m0T_sb = wk.tile([P, EPP], F8, tag="m0t")
                        nc.sync.dma_start(out=m0T_sb[:], in_=dram['m0T'][nt, :, :])
                        m0e_sb = wk.tile([P, EPP], F8, tag="m0e")
                        nc.sync.dma_start(out=m0e_sb[:], in_=dram['m0e'][nt, :, :])

                        acc = pacc.tile([P, HID + 4], F32, tag="acc")
                        NP2 = TE // 2
                        rps = []
                        for pr in range(NP2):
                            z2 = wk.tile([P, 2, HID], BF, tag="zzw", bufs=4)
                            ef_sb = wk.tile([P, 2, HID], BF, tag="ef", bufs=4)
                            nc.sync.dma_start(out=ef_sb[:],
                                              in_=dram['ef'][li, nt, pr, :, :])
                            for e in range(2):
                                et = pr * 2 + e
                                v = pv.tile([P, HID], F32, tag="v", bufs=2)
                                sl = slice(et * P, (et + 1) * P)
                                nc.tensor.matmul(v[:], m0T_sb[:, sl],
                                                 xr_rd[:, nt, :],
                                                 start=True, stop=False)
                                nc.tensor.matmul(v[:], cs['identbf'][:],
                                                 gbuf[:, et, :],
                                                 start=False, stop=False)
                                nc.tensor.matmul(v[:], cs['identbf'][:],
                                                 ef_sb[:, e, :],
                                                 start=False, stop=True)
                                nc.scalar.activation(z2[:, e, :], v[:],
                                                     AF.Prelu, alpha=0.2)
                            zw2 = wk.tile([P, 2, HID], BF, tag="zzw", bufs=4)
                            nc.vector.tensor_tensor(
                                zw2[:].rearrange("p e c -> p (e c)"),
                                z2[:].rearrange("p e c -> p (e c)"),
                                cs['attb2'][:, li, :], OP.mult)
                            al2 = wk.tile([P, 2 * H], F32, tag="al", bufs=NP2 + 2)
                            nc.vector.tensor_reduce(
                                al2[:], zw2[:].rearrange("p e (h c) -> p (e h) c", c=C),
                                mybir.AxisListType.X, OP.add)
                            rp2 = wk.tile([P, 2, HID + 4], BF, tag="rp", bufs=NP2 + 1)
                            nc.scalar.activation(
                                rp2[:, :, HID:HID + 4],
                                al2[:].rearrange("p (e h) -> p e h", h=H), AF.Exp)
                            rps.append(rp2)
                        for pr in range(NP2):
                            rp2 = rps[pr]
                            exb = rp2[:, :, HID:HID + 4].rearrange(
                                "p e (h o) -> p e h o", o=1).broadcast_to([P, 2, H, C])
                            nc.vector.tensor_tensor(
                                rp2[:, :, 0:HID].rearrange("p e (h c) -> p e h c", c=C),
                                gbuf[:, pr * 2:pr * 2 + 2, :].rearrange(
                                    "p e (h c) -> p e h c", c=C),
                                exb, OP.mult)
                        for et in range(TE):
                            nc.tensor.matmul(acc[:], m0e_sb[:, et * P:(et + 1) * P],
                                             rps[et // 2][:, et % 2, :],
                                             start=(et == 0), stop=(et == TE - 1))

                        pend_ep.append((nt, acc))
                        if len(pend_ep) > 1:
                            epilogue(li, *pend_ep.pop(0))
                    while pend_ep:
                        epilogue(li, *pend_ep.pop(0))

                h_cur, h_nxt = h_nxt, h_cur

            # ---- pooling + classifier
            with nc.named_scope("pool_cls"):
                gs = meta['gslot']
                pooled = {}
                for cc in range(2):
                    for op, nm in ((OP.add, 'sum'), (OP.max, 'max')):
                        r = wk.tile([P, GPC], F32, tag=f"pool_{nm}{cc}", bufs=1)
                        nc.vector.tensor_reduce(
                            r[:], hT[:, cc, :].rearrange("p (g s) -> p g s", s=gs),
                            mybir.AxisListType.X, op)
                        pooled[(nm, cc)] = r
                psA = pprod.tile([GPC, HID], F32, tag="prod")
                psB = pprod.tile([GPC, HID], F32, tag="prod")
                for cc in range(2):
                    nc.tensor.matmul(psA[:], pooled[('sum', cc)][:], cs['W1'][:, cc, :],
                                     start=(cc == 0), stop=(cc == 1))
                    nc.tensor.matmul(psB[:], pooled[('max', cc)][:], cs['W1'][:, 2 + cc, :],
                                     start=(cc == 0), stop=(cc == 1))
                z1a = wk.tile([GPC, HID], F32, tag="z1a", bufs=1)
                nc.vector.tensor_scalar(z1a[:], psA[:], cs['recip_cnt'][0:GPC, :],
                                        None, OP.mult)
                z1p = wk.tile([GPC, HID], F32, tag="z1p", bufs=1)
                nc.vector.tensor_tensor(z1p[:], z1a[:], psB[:], OP.add)
                u1 = wk.tile([GPC, HID], F32, tag="u1", bufs=1)
                nc.vector.tensor_tensor(u1[:], z1p[:], cs['s1bc'][0:GPC, :], OP.mult)
                u2 = wk.tile([GPC, HID], F32, tag="u2", bufs=1)
                nc.vector.scalar_tensor_tensor(
                    u2[:], u1[:], 1.0, cs['t1bc'][0:GPC, :], OP.mult, OP.add)
                z1f = wk.tile([GPC, HID], F32, tag="z1f", bufs=1)
                nc.vector.tensor_scalar(z1f[:], u2[:], 0.0, None, OP.max)

                z1T = wk.tile([P, 2, GPC], F32, tag="z1T", bufs=1)
                for cc in range(2):
                    tp = pprod.tile([P, GPC], F32, tag="tp", bufs=2)
                    nc.tensor.transpose(tp[:], z1f[:, cc * P:(cc + 1) * P],
                                        cs['identf'][0:GPC, 0:GPC])
                    nc.scalar.copy(z1T[:, cc, :], tp[:])
                z2ps = pprod.tile([GPC, HID // 2], F32, tag="prod")
                for cc in range(2):
                    nc.tensor.matmul(z2ps[:], z1T[:, cc, :], cs['W2'][:, cc, :],
                                     start=(cc == 0), stop=(cc == 1))
                v1 = wk.tile([GPC, HID // 2], F32, tag="v1", bufs=1)
                nc.vector.tensor_tensor(v1[:], z2ps[:], cs['s2bc'][0:GPC, :], OP.mult)
                v2 = wk.tile([GPC, HID // 2], F32, tag="v2", bufs=1)
                nc.vector.scalar_tensor_tensor(
                    v2[:], v1[:], 1.0, cs['t2bc'][0:GPC, :], OP.mult, OP.add)
                z2f = wk.tile([GPC, HID // 2], F32, tag="z2f", bufs=1)
                nc.vector.tensor_scalar(z2f[:], v2[:], 0.0, None, OP.max)
                tp2 = pprod.tile([P, GPC], F32, tag="tp", bufs=2)
                nc.tensor.transpose(tp2[:], z2f[:], cs['identf'][0:GPC, 0:GPC])
                z2T = wk.tile([P, GPC], F32, tag="z2T", bufs=1)
                nc.scalar.copy(z2T[:], tp2[:])
                z3ps = pprod.tile([GPC, 1], F32, tag="prod")
                nc.tensor.matmul(z3ps[:], z2T[:], cs['W3'][:], start=True, stop=True)
                osb = wk.tile([GPC, 1], F32, tag="osb", bufs=1)
                nc.vector.tensor_scalar(osb[:], z3ps[:], cs['b3'][0:GPC, :], None, OP.add)
                nc.sync.dma_start(out=out_d.ap(), in_=osb[:])

    nc.compile()
    return nc


# ---------------------------------------------------------------- entry point
_CACHE = {}
TRACE = False
LAST_EXEC_NS = None
LAST_RESULTS = None


def kernel(**inputs):
    global _T0
    _T0 = time.time()
    _log("preprocess start")
    per_core, meta, _gid = _preprocess(inputs)
    params = _fold_params(inputs)
    _log(f"preprocess done (meta={meta})")

    stage = os.environ.get('K_STAGE', 'full')
    key = (meta['npad'], meta['nt'], meta['te'], stage)
    if key not in _CACHE:
        _CACHE[key] = _build(meta, stage)
        _log(f"bass graph built+compiled (stage={stage})")
    nc = _CACHE[key]

    in_maps = []
    for c in range(NCORES):
        m = dict(params)
        m.update(per_core[c])
        in_maps.append(m)

    global LAST_EXEC_NS, LAST_RESULTS
    res = bass_utils.run_bass_kernel_spmd(nc, in_maps, core_ids=list(range(NCORES)),
                                          trace=TRACE)
    LAST_EXEC_NS = res.exec_time_ns
    LAST_RESULTS = res
    _log(f"hw run done exec_time_ns={res.exec_time_ns}")
    out = np.concatenate([res.results[c]['out'][:, 0] for c in range(NCORES)])
    return out.astype(np.float32)


if __name__ == "__main__":
    d = np.load("/root/problem/ref_data.npz")
    inputs = {k: d[k] for k in d.files if k != 'ref_out'}
    got = kernel(**inputs)
    ref = d['ref_out']
    rel = np.abs(got - ref).max() / np.abs(ref).max()
    print("rel err:", rel)



# revision 24
# speedup vs baseline: 1.0564x; 1.0564x over previous
"""Trainium2 8-core Bass kernel for the AntiviralGNN problem.

3-layer GATv2 (edge features, self-loops with mean fill) + mean/max graph
pooling + MLP classifier.  Nodes/edges are partitioned across 8 NeuronCores
by graph blocks (32 graphs per core); per layer each core computes its
xl = h@Wl+bl slice, all-gathers xl (bf16), then processes its incoming
edges: gathers xl[src] rows with dma_gather, reconstructs
z = leaky_relu(xl[src]+xr[dst]+ea@We) via TensorE matmuls (one-hot dst masks),
does the edge softmax without max-subtraction (alpha is provably in [-1,1])
and scatter-adds messages back to nodes with mask matmuls into PSUM.
"""

import os
import sys
import time

sys.path.insert(0, '/opt/trn_rl_repo')

import numpy as np

import concourse.bass as bass
import concourse.bacc as bacc
import concourse.mybir as mybir
import concourse.tile as tile
from concourse import bass_utils

BF16 = mybir.dt.np(mybir.dt.bfloat16)
FP8 = mybir.dt.np(mybir.dt.float8e4)
F8 = mybir.dt.float8e4
F32 = mybir.dt.float32
BF = mybir.dt.bfloat16
I16 = mybir.dt.int16
AF = mybir.ActivationFunctionType
OP = mybir.AluOpType

NCORES = 8
N, E, G = 20000, 320000, 256
FN, FE, HID, H, C = 64, 16, 256, 4, 64
GPC = G // NCORES          # graphs per core
EPS = 1e-5
P = 128


def _log(msg):
    print(f"[kernel +{time.time() - _T0:8.1f}s] {msg}", file=sys.stderr, flush=True)


_T0 = time.time()


def _chunk_bounds(nt_cnt):
    """Node-tile boundaries of the all-gather chunks (cumulative), unbalanced
    ~[40%, 30%, 20%, 10%] so the final (critical-path) chunk is small."""
    if nt_cnt < 4:
        return list(range(nt_cnt + 1))
    r4 = max(1, round(nt_cnt * 0.1))
    r3 = max(1, round(nt_cnt * 0.2))
    r2 = max(1, round(nt_cnt * 0.3))
    r1 = nt_cnt - r2 - r3 - r4
    return [0, r1, r1 + r2, r1 + r2 + r3, nt_cnt]


# ---------------------------------------------------------------- host prep
def _preprocess(inputs):
    x = inputs['x'].astype(np.float32)
    src, dst = inputs['edge_index'][0].astype(np.int64), inputs['edge_index'][1].astype(np.int64)
    batch = inputs['batch'].astype(np.int64)
    ea = inputs['edge_attr'].astype(np.float32)

    gstart = np.searchsorted(batch, np.arange(G + 1))
    gsize = np.diff(gstart)
    maxg = int(gsize.max())
    gslot = max(80, -(-maxg // 4) * 4)          # slots per graph, %4==0
    npad = GPC * gslot                           # padded nodes per core
    assert npad % P == 0
    nt_cnt = npad // P

    # global padded slot id per node.  All-gather chunks are UNBALANCED
    # (large first, small last) so the last chunk's collective - the only one
    # exposed on the critical path - is short.
    owner = batch // GPC
    slot_local = (batch % GPC) * gslot + (np.arange(N) - gstart[batch])
    cb = _chunk_bounds(nt_cnt)
    bs = np.array(cb) * P                        # chunk slot boundaries
    _k = np.searchsorted(bs, slot_local, 'right') - 1
    csz = bs[_k + 1] - bs[_k]
    gid = NCORES * bs[_k] + owner * csz + (slot_local - bs[_k])

    # self-loop attrs (mean of incoming)
    deg = np.zeros(N)
    np.add.at(deg, dst, 1.0)
    loop_attr = np.zeros((N, FE), np.float32)
    np.add.at(loop_attr, dst, ea)
    loop_attr /= np.maximum(deg, 1.0)[:, None].astype(np.float32)

    src2 = np.concatenate([src, np.arange(N)])
    dst2 = np.concatenate([dst, np.arange(N)])
    ea2 = np.concatenate([ea, loop_attr], axis=0)

    e_owner = owner[dst2]
    e_slot = slot_local[dst2]
    e_nt = e_slot // P
    e_rel = e_slot % P
    e_bin = e_owner * nt_cnt + e_nt

    order = np.argsort(e_bin, kind='stable')
    bin_cnt = np.bincount(e_bin, minlength=NCORES * nt_cnt)
    te = int(-(-bin_cnt.max() // P))             # edge tiles per node tile
    te += te % 2                                 # pair-batched loops need even te
    epp = te * P                                 # padded edges per node tile

    # position of each edge inside its padded bin
    bin_of = e_bin[order]
    start = np.zeros(NCORES * nt_cnt, np.int64)
    start[1:] = np.cumsum(bin_cnt)[:-1]
    pos_in_bin = np.arange(len(order)) - start[bin_of]
    flat = bin_of * epp + pos_in_bin             # position in padded global layout

    tot = NCORES * nt_cnt * epp
    src_g = np.zeros(tot, np.int64)              # gather ids (pad -> 0)
    rel_g = np.full(tot, 200.0, np.float32)      # dst_rel (pad -> 200)
    ea_g = np.zeros((tot, FE), np.float32)
    src_g[flat] = gid[src2[order]]
    rel_g[flat] = e_rel[order]
    ea_g[flat] = ea2[order]

    src_g = src_g.reshape(NCORES, nt_cnt, epp)
    rel_g = rel_g.reshape(NCORES, nt_cnt, epp)
    ea_g = ea_g.reshape(NCORES, nt_cnt, epp, FE)

    per_core = []
    for c in range(NCORES):
        # gather indices: per node tile, idx j -> [j%16, j//16]; replicated to
        # 128 partitions (one copy per Q7 core)
        for_nt = src_g[c].reshape(nt_cnt, epp)
        iw = for_nt.reshape(nt_cnt, epp // 16, 16).transpose(2, 0, 1).reshape(16, -1).astype(np.int16)
        idx_w = np.tile(iw, (8, 1))

        # ef = ea @ We per layer, packed for per-pair DMA [3, nt, te//2, 128, 512]
        WeT = inputs['conv_We'].astype(np.float32)          # [3, 16, 256]
        ef = np.einsum('nef,lfc->lnec', ea_g[c].reshape(nt_cnt, epp, FE), WeT)
        ef = ef.reshape(3, nt_cnt, te // 2, 2, P, HID).transpose(0, 1, 2, 4, 3, 5)
        ef = ef.reshape(3, nt_cnt, te // 2, P, 2 * HID).astype(FP8)

        # m0T [nt, 128, te*128]: (n == rel)
        rel_tiles = rel_g[c].reshape(nt_cnt, epp)
        m0T = (np.arange(P)[None, :, None] == rel_tiles[:, None, :]).astype(np.float32)
        # m0e [nt, 128e, te*128]: row e, col et*128+n = (rel[et*128+e] == n)
        rel_etp = rel_g[c].reshape(nt_cnt, te, P)
        m0e = (rel_etp[:, :, :, None] == np.arange(P)[None, None, None, :])
        m0e = m0e.transpose(0, 2, 1, 3).reshape(nt_cnt, P, epp).astype(np.float32)

        # node mask [128, nt]
        msk = np.zeros(npad, np.float32)
        msk[slot_local[owner == c]] = 1.0
        msk = msk.reshape(nt_cnt, P).T.copy()

        # x transposed with ones row [65, npad]
        xT = np.zeros((FN + 1, npad), np.float32)
        xT[:FN, slot_local[owner == c]] = x[owner == c].T
        xT[FN, :] = 1.0
        xT = xT.astype(BF16)

        cnt = gsize[c * GPC:(c + 1) * GPC]
        rc = np.zeros((P, 1), np.float32)
        rc[:GPC, 0] = 1.0 / cnt
        per_core.append(dict(idx=idx_w, ef=ef,
                             m0T=m0T.astype(FP8), m0e=m0e.astype(FP8),
                             mask=msk, xT=xT, recip_cnt=rc))

    meta = dict(gslot=gslot, npad=npad, nt=nt_cnt, te=te)
    return per_core, meta, gid


def _fold_params(inputs):
    """Host-side parameter folding -> device tensors (shared across cores)."""
    p = {}
    encW = np.concatenate([inputs['enc_W'], inputs['enc_b'][None, :]], axis=0)
    p['encW'] = encW.astype(BF16)                                   # [65, 256]

    Wl = inputs['conv_Wl'].reshape(3, 2, P, HID)                    # [3,2,128,256]
    Wr = inputs['conv_Wr'].reshape(3, 2, P, HID)
    Wlr = np.concatenate([Wl, Wr], axis=3)                          # [3,2,128,512]
    p['Wlr'] = Wlr.transpose(2, 0, 1, 3).astype(np.float16)         # [128,3,2,512]
    p['We'] = inputs['conv_We'].transpose(1, 0, 2).astype(BF16)     # [16,3,256]

    att = inputs['conv_att'].reshape(3, 1, HID)
    att2 = np.concatenate([att, att], axis=2)
    p['attb2'] = np.tile(att2, (1, P, 1)).transpose(1, 0, 2).astype(BF16)  # [128,3,512]

    def bc(v):   # [3,256] -> [128,3,256]
        return np.tile(v[:, None, :], (1, P, 1)).transpose(1, 0, 2).astype(np.float32)

    p['blbc'] = bc(inputs['conv_bl'])
    p['brbc'] = bc(inputs['conv_br'])
    rs = 1.0 / np.sqrt(inputs['bn_var'] + EPS)
    sc = rs * inputs['bn_gamma']
    sh = (inputs['conv_bias'] - inputs['bn_mean']) * sc + inputs['bn_beta']
    p['bnsc'] = bc(sc)
    p['bnsh'] = bc(sh)

    p['identbf'] = np.eye(P, dtype=np.float32).astype(BF16)
    p['identf'] = np.eye(P, dtype=np.float32)

    p['W1'] = inputs['cls_W1'].reshape(4, P, HID).transpose(1, 0, 2).astype(np.float32)  # [128,4,256]
    p['W2'] = inputs['cls_W2'].reshape(2, P, HID // 2).transpose(1, 0, 2).astype(np.float32)  # [128,2,128]
    p['W3'] = inputs['cls_W3'].astype(np.float32)                   # [128,1]

    s1 = inputs['cls_g1'] / np.sqrt(inputs['cls_v1'] + EPS)
    t1 = (inputs['cls_b1'] - inputs['cls_m1']) * s1 + inputs['cls_bt1']
    s2 = inputs['cls_g2'] / np.sqrt(inputs['cls_v2'] + EPS)
    t2 = (inputs['cls_b2'] - inputs['cls_m2']) * s2 + inputs['cls_bt2']
    p['s1bc'] = np.tile(s1, (P, 1)).astype(np.float32)              # [128,256]
    p['t1bc'] = np.tile(t1, (P, 1)).astype(np.float32)
    p['s2bc'] = np.tile(s2, (P, 1)).astype(np.float32)              # [128,128]
    p['t2bc'] = np.tile(t2, (P, 1)).astype(np.float32)
    p['b3'] = np.tile(inputs['cls_b3'].reshape(1, 1), (P, 1)).astype(np.float32)
    return p


# ---------------------------------------------------------------- device graph
def _build(meta, stage='full'):
    npad, NT, TE = meta['npad'], meta['nt'], meta['te']
    EPP = TE * P
    nc = bacc.Bacc("TRN2", target_bir_lowering=False, debug=False, num_devices=NCORES,
                   num_swdge_queues=2)

    dram = {}

    def din(name, shape, dt):
        dram[name] = nc.dram_tensor(name, shape, dt, kind="ExternalInput")
        return dram[name]

    din('xT', [FN + 1, npad], BF)
    din('idx', [P, NT * TE * 8], I16)
    din('ef', [3, NT, TE // 2, P, 2 * HID], F8)
    din('m0T', [NT, P, EPP], F8)
    din('m0e', [NT, P, EPP], F8)
    din('mask', [P, NT], F32)
    din('recip_cnt', [P, 1], F32)
    din('encW', [FN + 1, HID], BF)
    din('Wlr', [P, 3, 2, 2 * HID], mybir.dt.float16)
    din('We', [FE, 3, HID], BF)
    din('attb2', [P, 3, 2 * HID], BF)
    din('blbc', [P, 3, HID], F32)
    din('brbc', [P, 3, HID], F32)
    din('bnsc', [P, 3, HID], F32)
    din('bnsh', [P, 3, HID], F32)
    din('identbf', [P, P], BF)
    din('identf', [P, P], F32)
    din('W1', [P, 4, HID], F32)
    din('W2', [P, 2, HID // 2], F32)
    din('W3', [P, 1], F32)
    din('s1bc', [P, HID], F32)
    din('t1bc', [P, HID], F32)
    din('s2bc', [P, HID // 2], F32)
    din('t2bc', [P, HID // 2], F32)
    din('b3', [P, 1], F32)
    out_d = nc.dram_tensor("out", [GPC, 1], F32, kind="ExternalOutput")


    with tile.TileContext(nc) as tc:
        with (
            tc.tile_pool(name="const", bufs=1) as cp,
            tc.tile_pool(name="state", bufs=1) as st,
            tc.tile_pool(name="work", bufs=2) as wk,
            tc.tile_pool(name="pv", bufs=2, space="PSUM") as pv,
            tc.tile_pool(name="pacc", bufs=2, space="PSUM") as pacc,
            tc.tile_pool(name="pprod", bufs=2, space="PSUM") as pprod,
            tc.tile_pool(name="dramp", bufs=2, space="DRAM") as dp,
        ):
            # ---- load constants
            cs = {}
            for name in ['xT', 'idx', 'mask', 'recip_cnt', 'encW', 'Wlr', 'We', 'attb2', 'blbc', 'brbc', 'bnsc', 'bnsh',
                         'identbf', 'identf', 'W1', 'W2', 'W3', 's1bc',
                         't1bc', 's2bc', 't2bc', 'b3']:
                d = dram[name]
                t = cp.tile(list(d.shape), d.dtype, name=f"c_{name}")
                nc.sync.dma_start(out=t[:], in_=d.ap())
                cs[name] = t

            # ---- state
            h_a = st.tile([P, NT, HID], F32, name="h_a")
            h_b = st.tile([P, NT, HID], F32, name="h_b")
            hT_a = st.tile([P, 2, npad], F32, name="hT_a")
            hTb = st.tile([P, 2, npad], mybir.dt.float16, name="hTb")
            xr_all = st.tile([P, NT, HID], BF, name="xr_all")

            def make_hT(hT, h_cur, nt):
                for cc in range(2):
                    tp = pprod.tile([P, P], F32, tag="tp", bufs=2)
                    nc.tensor.transpose(tp[:], h_cur[:, nt, cc * P:(cc + 1) * P], cs['identf'][:])
                    nc.scalar.copy(hT[:, cc, nt * P:(nt + 1) * P], tp[:])
                    nc.scalar.copy(hTb[:, cc, nt * P:(nt + 1) * P], tp[:])

            # chunked all-gather, unbalanced (large->small) chunks; tile bounds
            CB = _chunk_bounds(NT)
            NCH = len(CB) - 1
            CH_ENDS = {CB[k + 1]: k for k in range(NCH)}
            hT = hT_a
            xr_b = st.tile([P, NT, HID], BF, name="xr_b")
            xr_ab = [xr_all, xr_b]

            def produce(li, nt, xl_bounce):
                pp = pprod.tile([P, 2 * HID], F32, tag="prod")
                for cc in range(2):
                    nc.tensor.matmul(pp[:], hTb[:, cc, nt * P:(nt + 1) * P],
                                     cs['Wlr'][:, li, cc, :],
                                     start=(cc == 0), stop=(cc == 1))
                xl_sb = wk.tile([P, HID], BF, tag="xlsb")
                nc.vector.scalar_tensor_tensor(
                    xl_sb[:], pp[:, 0:HID], 1.0, cs['blbc'][:, li, :], OP.mult, OP.add)
                nc.scalar.dma_start(out=xl_bounce[nt * P:(nt + 1) * P, :], in_=xl_sb[:])
                nc.vector.scalar_tensor_tensor(
                    xr_ab[li % 2][:, nt, :], pp[:, HID:2 * HID], 1.0,
                    cs['brbc'][:, li, :], OP.mult, OP.add)

            def chunk_collective(xl_bounce, xl_full, ck):
                rs, re = CB[ck] * P, CB[ck + 1] * P
                nc.gpsimd.collective_compute(
                    "AllGather", OP.bypass,
                    replica_groups=[list(range(NCORES))],
                    ins=[xl_bounce[rs:re, :].opt()],
                    outs=[xl_full[NCORES * rs:NCORES * re, :].opt()],
                )

            xlb = [None] * 3
            xlf = [None] * 3
            xlb[0] = dp.tile([npad, HID], BF, tag="xlb", name="xlb0")
            xlf[0] = dp.tile([NCORES * npad, HID], BF, tag="xlf", name="xlf0")

            # ---- encoder (+ layer-0 production, chunk-collectives)
            with nc.named_scope("encoder"):
                for nt in range(NT):
                    pp = pprod.tile([P, HID], F32, tag="prod")
                    nc.tensor.matmul(pp[:], cs['xT'][:, nt * P:(nt + 1) * P],
                                     cs['encW'][:], start=True, stop=True)
                    nc.scalar.activation(h_a[:, nt, :], pp[:], AF.Relu,
                                         scale=cs['mask'][:, nt:nt + 1])
                    make_hT(hT, h_a, nt)
                    produce(0, nt, xlb[0])
                    if (nt + 1) in CH_ENDS:
                        chunk_collective(xlb[0], xlf[0], CH_ENDS[nt + 1])

            h_cur, h_nxt = h_a, h_b

            # ---- layers
            for li in range(3):
                if li < 2:
                    xlb[li + 1] = dp.tile([npad, HID], BF, tag="xlb",
                                          name=f"xlb{li + 1}")
                    xlf[li + 1] = dp.tile([NCORES * npad, HID], BF, tag="xlf",
                                          name=f"xlf{li + 1}")
                xl_full = xlf[li]
                xr_rd = xr_ab[li % 2]
                pend_ep = []

                def epilogue(li, nt, acc):
                    den = wk.tile([P, H], F32, tag="den")
                    nc.vector.tensor_scalar(den[:], acc[:, HID:HID + 4], 1e-30,
                                            None, OP.max)
                    rec = wk.tile([P, H], F32, tag="rec")
                    nc.vector.reciprocal(rec[:], den[:])
                    hc = wk.tile([P, HID], F32, tag="ep", bufs=2)
                    for hh in range(H):
                        nc.scalar.activation(hc[:, hh * C:(hh + 1) * C],
                                             acc[:, hh * C:(hh + 1) * C],
                                             AF.Copy, scale=rec[:, hh:hh + 1])
                    t1 = wk.tile([P, HID], F32, tag="ep", bufs=2)
                    nc.vector.tensor_tensor(t1[:], hc[:], cs['bnsc'][:, li, :], OP.mult)
                    t2 = wk.tile([P, HID], F32, tag="ep", bufs=2)
                    nc.vector.scalar_tensor_tensor(
                        t2[:], t1[:], 1.0, cs['bnsh'][:, li, :], OP.mult, OP.add)
                    t3 = wk.tile([P, HID], F32, tag="ep", bufs=2)
                    nc.scalar.activation(t3[:], t2[:], AF.Relu)
                    nc.vector.scalar_tensor_tensor(
                        h_nxt[:, nt, :], t3[:], cs['mask'][:, nt:nt + 1],
                        h_cur[:, nt, :], OP.mult, OP.add)
                    make_hT(hT, h_nxt, nt)
                    if li < 2:
                        produce(li + 1, nt, xlb[li + 1])
                        if (nt + 1) in CH_ENDS:
                            chunk_collective(xlb[li + 1], xlf[li + 1],
                                             CH_ENDS[nt + 1])
                with nc.named_scope(f"layer{li}_edges"):
                    for nt in range(NT):
                        gbuf = wk.tile([P, TE, HID], BF, tag="gbuf", bufs=3)
                        th = TE // 2
                        nc.gpsimd.dma_gather(
                            gbuf[:, 0:th, :], xl_full[:],
                            cs['idx'][:, nt * TE * 8:nt * TE * 8 + th * 8],
                            th * P, th * P, HID, single_packet=False, queue_num=0)
                        nc.gpsimd.dma_gather(
                            gbuf[:, th:TE, :], xl_full[:],
                            cs['idx'][:, nt * TE * 8 + th * 8:(nt + 1) * TE * 8],
                            (TE - th) * P, (TE - th) * P, HID, single_packet=False,
                            queue_num=1)
                        m0T_sb = wk.tile([P, EPP], F8, tag="m0t")
                        nc.sync.dma_start(out=m0T_sb[:], in_=dram['m0T'][nt, :, :])
                        m0e_sb = wk.tile([P, EPP], F8, tag="m0e")
                        nc.sync.dma_start(out=m0e_sb[:], in_=dram['m0e'][nt, :, :])

                        acc = pacc.tile([P, HID + 4], F32, tag="acc")
                        NP2 = TE // 2
                        rps = []
                        for pr in range(NP2):
                            z2 = wk.tile([P, 2, HID], BF, tag="zzw", bufs=4)
                            ef_sb = wk.tile([P, 2, HID], F8, tag="ef", bufs=4)
                            nc.scalar.dma_start(out=ef_sb[:],
                                                in_=dram['ef'][li, nt, pr, :, :])
                            for e in range(2):
                                et = pr * 2 + e
                                v = pv.tile([P, HID], F32, tag="v", bufs=2)
                                sl = slice(et * P, (et + 1) * P)
                                nc.tensor.matmul(v[:], m0T_sb[:, sl],
                                                 xr_rd[:, nt, :],
                                                 start=True, stop=False)
                                nc.tensor.matmul(v[:], cs['identbf'][:],
                                                 gbuf[:, et, :],
                                                 start=False, stop=False)
                                nc.tensor.matmul(v[:], cs['identbf'][:],
                                                 ef_sb[:, e, :],
                                                 start=False, stop=True)
                                nc.scalar.activation(z2[:, e, :], v[:],
                                                     AF.Prelu, alpha=0.2)
                            zw2 = wk.tile([P, 2, HID], BF, tag="zzw", bufs=4)
                            nc.vector.tensor_tensor(
                                zw2[:].rearrange("p e c -> p (e c)"),
                                z2[:].rearrange("p e c -> p (e c)"),
                                cs['attb2'][:, li, :], OP.mult)
                            al2 = wk.tile([P, 2 * H], F32, tag="al", bufs=NP2 + 2)
                            nc.vector.tensor_reduce(
                                al2[:], zw2[:].rearrange("p e (h c) -> p (e h) c", c=C),
                                mybir.AxisListType.X, OP.add)
                            rp2 = wk.tile([P, 2, HID + 4], BF, tag="rp", bufs=NP2 + 1)
                            nc.scalar.activation(
                                rp2[:, :, HID:HID + 4],
                                al2[:].rearrange("p (e h) -> p e h", h=H), AF.Exp)
                            rps.append(rp2)
                        for pr in range(NP2):
                            rp2 = rps[pr]
                            exb = rp2[:, :, HID:HID + 4].rearrange(
                                "p e (h o) -> p e h o", o=1).broadcast_to([P, 2, H, C])
                            nc.vector.tensor_tensor(
                                rp2[:, :, 0:HID].rearrange("p e (h c) -> p e h c", c=C),
                                gbuf[:, pr * 2:pr * 2 + 2, :].rearrange(
                                    "p e (h c) -> p e h c", c=C),
                                exb, OP.mult)
                        for et in range(TE):
                            nc.tensor.matmul(acc[:], m0e_sb[:, et * P:(et + 1) * P],
                                             rps[et // 2][:, et % 2, :],
                                             start=(et == 0), stop=(et == TE - 1))

                        pend_ep.append((nt, acc))
                        if len(pend_ep) > 1:
                            epilogue(li, *pend_ep.pop(0))
                    while pend_ep:
                        epilogue(li, *pend_ep.pop(0))

                h_cur, h_nxt = h_nxt, h_cur

            # ---- pooling + classifier
            with nc.named_scope("pool_cls"):
                gs = meta['gslot']
                pooled = {}
                for cc in range(2):
                    for op, nm in ((OP.add, 'sum'), (OP.max, 'max')):
                        r = wk.tile([P, GPC], F32, tag=f"pool_{nm}{cc}", bufs=1)
                        nc.vector.tensor_reduce(
                            r[:], hT[:, cc, :].rearrange("p (g s) -> p g s", s=gs),
                            mybir.AxisListType.X, op)
                        pooled[(nm, cc)] = r
                psA = pprod.tile([GPC, HID], F32, tag="prod")
                psB = pprod.tile([GPC, HID], F32, tag="prod")
                for cc in range(2):
                    nc.tensor.matmul(psA[:], pooled[('sum', cc)][:], cs['W1'][:, cc, :],
                                     start=(cc == 0), stop=(cc == 1))
                    nc.tensor.matmul(psB[:], pooled[('max', cc)][:], cs['W1'][:, 2 + cc, :],
                                     start=(cc == 0), stop=(cc == 1))
                z1a = wk.tile([GPC, HID], F32, tag="z1a", bufs=1)
                nc.vector.tensor_scalar(z1a[:], psA[:], cs['recip_cnt'][0:GPC, :],
                                        None, OP.mult)
                z1p = wk.tile([GPC, HID], F32, tag="z1p", bufs=1)
                nc.vector.tensor_tensor(z1p[:], z1a[:], psB[:], OP.add)
                u1 = wk.tile([GPC, HID], F32, tag="u1", bufs=1)
                nc.vector.tensor_tensor(u1[:], z1p[:], cs['s1bc'][0:GPC, :], OP.mult)
                u2 = wk.tile([GPC, HID], F32, tag="u2", bufs=1)
                nc.vector.scalar_tensor_tensor(
                    u2[:], u1[:], 1.0, cs['t1bc'][0:GPC, :], OP.mult, OP.add)
                z1f = wk.tile([GPC, HID], F32, tag="z1f", bufs=1)
                nc.vector.tensor_scalar(z1f[:], u2[:], 0.0, None, OP.max)

                z1T = wk.tile([P, 2, GPC], F32, tag="z1T", bufs=1)
                for cc in range(2):
                    tp = pprod.tile([P, GPC], F32, tag="tp", bufs=2)
                    nc.tensor.transpose(tp[:], z1f[:, cc * P:(cc + 1) * P],
                                        cs['identf'][0:GPC, 0:GPC])
                    nc.scalar.copy(z1T[:, cc, :], tp[:])
                z2ps = pprod.tile([GPC, HID // 2], F32, tag="prod")
                for cc in range(2):
                    nc.tensor.matmul(z2ps[:], z1T[:, cc, :], cs['W2'][:, cc, :],
                                     start=(cc == 0), stop=(cc == 1))
                v1 = wk.tile([GPC, HID // 2], F32, tag="v1", bufs=1)
                nc.vector.tensor_tensor(v1[:], z2ps[:], cs['s2bc'][0:GPC, :], OP.mult)
                v2 = wk.tile([GPC, HID // 2], F32, tag="v2", bufs=1)
                nc.vector.scalar_tensor_tensor(
                    v2[:], v1[:], 1.0, cs['t2bc'][0:GPC, :], OP.mult, OP.add)
                z2f = wk.tile([GPC, HID // 2], F32, tag="z2f", bufs=1)
                nc.vector.tensor_scalar(z2f[:], v2[:], 0.0, None, OP.max)
                tp2 = pprod.tile([P, GPC], F32, tag="tp", bufs=2)
                nc.tensor.transpose(tp2[:], z2f[:], cs['identf'][0:GPC, 0:GPC])
                z2T = wk.tile([P, GPC], F32, tag="z2T", bufs=1)
                nc.scalar.copy(z2T[:], tp2[:])
                z3ps = pprod.tile([GPC, 1], F32, tag="prod")
                nc.tensor.matmul(z3ps[:], z2T[:], cs['W3'][:], start=True, stop=True)
                osb = wk.tile([GPC, 1], F32, tag="osb", bufs=1)
                nc.vector.tensor_scalar(osb[:], z3ps[:], cs['b3'][0:GPC, :], None, OP.add)
                nc.sync.dma_start(out=out_d.ap(), in_=osb[:])

    nc.compile()
    return nc


# ---------------------------------------------------------------- entry point
_CACHE = {}
TRACE = False
LAST_EXEC_NS = None
LAST_RESULTS = None


def kernel(**inputs):
    global _T0
    _T0 = time.time()
    _log("preprocess start")
    per_core, meta, _gid = _preprocess(inputs)
    params = _fold_params(inputs)
    _log(f"preprocess done (meta={meta})")

    stage = os.environ.get('K_STAGE', 'full')
    key = (meta['npad'], meta['nt'], meta['te'], stage)
    if key not in _CACHE:
        _CACHE[key] = _build(meta, stage)
        _log(f"bass graph built+compiled (stage={stage})")
    nc = _CACHE[key]

    in_maps = []
    for c in range(NCORES):
        m = dict(params)
        m.update(per_core[c])
        in_maps.append(m)

    global LAST_EXEC_NS, LAST_RESULTS
    res = bass_utils.run_bass_kernel_spmd(nc, in_maps, core_ids=list(range(NCORES)),
                                          trace=TRACE)
    LAST_EXEC_NS = res.exec_time_ns
    LAST_RESULTS = res
    _log(f"hw run done exec_time_ns={res.exec_time_ns}")
    out = np.concatenate([res.results[c]['out'][:, 0] for c in range(NCORES)])
    return out.astype(np.float32)


if __name__ == "__main__":
    d = np.load("/root/problem/ref_data.npz")
    inputs = {k: d[k] for k in d.files if k != 'ref_out'}
    got = kernel(**inputs)
    ref = d['ref_out']
    rel = np.abs(got - ref).max() / np.abs(ref).max()
    print("rel err:", rel)

